# revision 37
# baseline (speedup 1.0000x reference)
"""Bass/Tile TRN2 kernel for nn_BertSelfAttention2 (B=2, S=2048, D=1024, H=16).

Sharding: 8 cores = 2 (batch) x 4 (head groups of 4 heads). Each core
computes Q/K projections for its 4 heads (as 2 packed pairs), the modified
attention (kt = softplus(k), v = q + k, mask on the query axis), and writes
its [S, 256] slice of the output.

v2 layout/engine plan:
- All matmul operands in bf16 (PSUM accumulation stays fp32); X^T, W
  shipped pre-transposed in bf16 so DMA bytes halve and FWL kicks in.
- kt = softplus(k) = ln(1 + e^k) via ACT Exp then Ln(.+1), but PHASE
  GROUPED: all 8 Exp acts, then all 8 Ln acts, then the attention Exp
  block -> 3 ACT_TABLE_LOADs total instead of 17 (the table pass
  assigns Exp/Ln to different sets and reloads ~1.3us at every
  function boundary). The softplus work lands in the projection phase
  where the ACT engine is otherwise idle.
- Scores/probs stay in "T" orientation (scoresT[k, q]); the query-axis
  mask is applied by zeroing masked query columns of Q (softmax of an
  all-zero score column reproduces the reference's uniform-probability
  behaviour exactly). Softmax denominators come from a ones-column in V'.
- exp supertiles are [128, 1536]/[128, 1024] (3-2 key-chunk groups) to
  amortize the ~220ns per-activation overhead; sA/sB take 6 PSUM banks
  (the proj accumulators reuse those banks via tags), cA/cB take 2.
"""
import sys

if "/opt/trn_rl_repo" not in sys.path:
    sys.path.insert(0, "/opt/trn_rl_repo")

import numpy as np
import ml_dtypes

B, S, D = 2, 2048, 1024
H = 16
HD = 64
NCORES = 8
HPC = H // (NCORES // B)     # heads per core = 4
NG = HPC // 2                # head-pair groups per core = 2
SC = 4                       # 512-wide s chunks
KC = S // 128                # 16 key chunks
SUPERS = [(0, 3), (3, 3), (6, 3), (9, 3), (12, 2), (14, 2)]

_CACHE = {}


def _build():
    import concourse.tile as tile
    from concourse import bacc, mybir
    from concourse.masks import make_identity
    from concourse.alu_op_type import AluOpType
    from concourse.tile import add_dep_helper

    F32 = mybir.dt.float32
    BF16 = mybir.dt.bfloat16
    AF = mybir.ActivationFunctionType

    nc = bacc.Bacc(None, target_bir_lowering=False, debug=False)

    xt = nc.declare_dram_parameter("xt", [8 * 128, S], BF16, isOutput=False)
    wq = nc.declare_dram_parameter("wq", [128, NG * 8 * 128], BF16, isOutput=False)
    wk = nc.declare_dram_parameter("wk", [128, NG * 8 * 128], BF16, isOutput=False)
    bq = nc.declare_dram_parameter("bq", [2 * 128], F32, isOutput=False)
    bk = nc.declare_dram_parameter("bk", [2 * 128], F32, isOutput=False)
    maskb = nc.declare_dram_parameter("maskb", [1, S], BF16, isOutput=False)
    out = nc.declare_dram_parameter("out", [NG * S, 128], F32, isOutput=True)

    with tile.TileContext(nc) as tc, \
         nc.allow_low_precision(reason="bf16 pipeline; validated vs fp32 "
                                "reference at rel tol 2e-2"):
        with tc.tile_pool(name="consts", bufs=1) as consts, \
             tc.tile_pool(name="big", bufs=1) as big, \
             tc.tile_pool(name="tmp", bufs=2) as tmp, \
             tc.tile_pool(name="expp", bufs=3) as expp, \
             tc.tile_pool(name="ep", bufs=2) as ep, \
             tc.tile_pool(name="ps_s", bufs=1, space="PSUM") as ps_s, \
             tc.tile_pool(name="ps_c", bufs=1, space="PSUM") as ps_c:

            identb = consts.tile([128, 128], BF16)
            make_identity(nc, identb)

            # weights for both groups in one tile each; slice per (g, dc).
            # group-0 halves land first so the first projection matmul can
            # start before the X^T tiles finish streaming in
            wq_t = consts.tile([128, NG * 8 * 128], BF16, name="wq_t")
            wk_t = consts.tile([128, NG * 8 * 128], BF16, name="wk_t")
            nc.gpsimd.dma_start(out=wq_t[:, 0:1024], in_=wq[:, 0:1024])
            nc.gpsimd.dma_start(out=wk_t[:, 0:1024], in_=wk[:, 0:1024])

            bq_t, bk_t = [], []
            for g in range(NG):
                for lst, par, nm in ((bq_t, bq, "bq"), (bk_t, bk, "bk")):
                    t = consts.tile([128, 1], F32, name=f"{nm}{g}")
                    nc.gpsimd.dma_start(
                        out=t,
                        in_=par[g * 128:(g + 1) * 128].rearrange(
                            "(p o) -> p o", o=1))
                    lst.append(t)

            # X^T as 8 [128, 2048] bf16 tiles (one per D-chunk), streamed in
            # two column halves so the first projection chunk (which needs
            # ALL 8 D-chunks) is ready after ~half the X bytes
            xt_t = [consts.tile([128, S], BF16, name=f"xt{dc}")
                    for dc in range(8)]
            for dc in range(8):
                nc.sync.dma_start(out=xt_t[dc][:, 0:1024],
                                  in_=xt[dc * 128:(dc + 1) * 128, 0:1024])
            nc.gpsimd.dma_start(out=wq_t[:, 1024:2048], in_=wq[:, 1024:2048])
            nc.gpsimd.dma_start(out=wk_t[:, 1024:2048], in_=wk[:, 1024:2048])
            for dc in range(8):
                nc.sync.dma_start(out=xt_t[dc][:, 1024:2048],
                                  in_=xt[dc * 128:(dc + 1) * 128, 1024:2048])

            mask_row = consts.tile([1, S], BF16)
            nc.gpsimd.dma_start(out=mask_row, in_=maskb[:, :])
            mask_t = consts.tile([128, S], BF16, name="mask_t")
            for scc in range(SC):
                nc.gpsimd.partition_broadcast(
                    mask_t[:, scc * 512:(scc + 1) * 512],
                    mask_row[0:1, scc * 512:(scc + 1) * 512])

            # persistent activations (bf16):
            # qtp[g][hh]: masked Q^T for head hh of pair g, other head's rows 0
            # kt[g]:      softplus(K^T), both heads packed
            # vp[h]:      V' chunks [128 keys, 64 dims + ones col] x 16, packed
            qtp = [[big.tile([128, S], BF16, name=f"qtp{g}_{hh}")
                    for hh in range(2)] for g in range(NG)]
            kt = [big.tile([128, S], BF16, name=f"kt{g}") for g in range(NG)]
            vp = [big.tile([128, KC * 65], BF16, name=f"vp{h}")
                  for h in range(HPC)]

            for g in range(NG):
                nc.vector.memset(qtp[g][0][64:128, :], 0.0)
                nc.vector.memset(qtp[g][1][0:64, :], 0.0)
            for h in range(HPC):
                nc.vector.memset(
                    vp[h].rearrange("p (k o) -> p k o", o=65)[:, :, 64:65],
                    1.0)

            def emit_vtrans(g, sc, vts):
                for hh in range(2):
                    h = g * 2 + hh
                    hsl = slice(hh * 64, (hh + 1) * 64)
                    for jj in range(4):
                        j = sc * 4 + jj
                        pv = ps_c.tile([128, 64], BF16, tag="cA" if hh == 0
                                       else "cB", name=f"pv{g}_{hh}_{j}")
                        nc.tensor.transpose(pv,
                                            vts[hsl, jj * 128:(jj + 1) * 128],
                                            identb[hsl, hsl])
                        nc.vector.tensor_copy(vp[h][:, j * 65:j * 65 + 64], pv)

            def proj_group(g, vts_hist):
                # s-chunk PAIRS share one accumulator tile per tag so the PE
                # gets a 32-matmul continuous run per pair (the single-buffer
                # per-chunk version interleaved PE<->DVE waits, which kept
                # resetting the PE clock ramp and ran proj at ~1.2GHz)
                for scp in range(SC // 2):
                    pqp = ps_s.tile([128, 1024], F32, tag="sA",
                                    name=f"pqp{g}_{scp}")
                    pkp = ps_s.tile([128, 1024], F32, tag="sB",
                                    name=f"pkp{g}_{scp}")
                    for lsc in range(2):
                        sc = scp * 2 + lsc
                        ssl = slice(sc * 512, (sc + 1) * 512)
                        psl = slice(lsc * 512, (lsc + 1) * 512)
                        for dc in range(8):
                            nc.tensor.matmul(
                                pqp[:, psl],
                                wq_t[:, g * 1024 + dc * 128:g * 1024 + (dc + 1) * 128],
                                xt_t[dc][:, ssl], start=(dc == 0), stop=(dc == 7))
                    for lsc in range(2):
                        sc = scp * 2 + lsc
                        ssl = slice(sc * 512, (sc + 1) * 512)
                        psl = slice(lsc * 512, (lsc + 1) * 512)
                        for dc in range(8):
                            nc.tensor.matmul(
                                pkp[:, psl],
                                wk_t[:, g * 1024 + dc * 128:g * 1024 + (dc + 1) * 128],
                                xt_t[dc][:, ssl], start=(dc == 0), stop=(dc == 7))
                    for lsc in range(2):
                        sc = scp * 2 + lsc
                        ssl = slice(sc * 512, (sc + 1) * 512)
                        psl = slice(lsc * 512, (lsc + 1) * 512)
                        pq = pqp[:, psl]
                        pk = pkp[:, psl]
                        # stage q+bq to SBUF bf16 (one PSUM input per DVE
                        # op); k+bk never materializes: the te-Exp and the
                        # v-add both fold the bias in while reading PSUM
                        tq = tmp.tile([128, 512], BF16, tag="tq",
                                      name=f"tq{g}_{sc}")
                        nc.vector.tensor_scalar_add(tq, pq, bq_t[g])
                        vts = tmp.tile([128, 512], BF16, tag="vts",
                                       name=f"vts{g}_{sc}")
                        nc.vector.scalar_tensor_tensor(
                            out=vts, in0=pk, scalar=bk_t[g], in1=tq,
                            op0=AluOpType.add, op1=AluOpType.add)
                        # masked q per head
                        nc.vector.tensor_mul(qtp[g][0][0:64, ssl], tq[0:64, :],
                                             mask_t[0:64, ssl])
                        nc.vector.tensor_mul(qtp[g][1][64:128, ssl],
                                             tq[64:128, :],
                                             mask_t[64:128, ssl])
                        # te = e^(k+bk) now (Exp block, straight from PSUM);
                        # kt = Ln(te+1) later so the ACT engine never
                        # alternates tables mid-phase
                        te = big.tile([128, 512], BF16, name=f"te{g}_{sc}")
                        tei = nc.scalar.activation(out=te, in_=pk,
                                                   func=AF.Exp, bias=bk_t[g])
                        te_hist.append((g, sc, te, tei))
                        vts_hist.append((g, sc, vts))
                        # V' transposes run one chunk behind so the PE is
                        # never gated on this chunk's DVE chain
                        if len(vts_hist) > 1:
                            emit_vtrans(*vts_hist[-2])
                return vts_hist

            def attn_group(g):
                for qc in range(SC):
                    qsl = slice(qc * 512, (qc + 1) * 512)
                    cA = ps_c.tile([65, 512], F32, tag="cA", name=f"cA{g}_{qc}")
                    cB = ps_c.tile([65, 512], F32, tag="cB", name=f"cB{g}_{qc}")
                    for kc0, ns in SUPERS:
                        sA = ps_s.tile([128, ns * 512], F32, tag="sA",
                                       name=f"sA{g}_{qc}_{kc0}")
                        sB = ps_s.tile([128, ns * 512], F32, tag="sB",
                                       name=f"sB{g}_{qc}_{kc0}")
                        for kk in range(ns):
                            kc = kc0 + kk
                            osl = slice(kk * 512, (kk + 1) * 512)
                            lhs = kt[g][:, kc * 128:(kc + 1) * 128]
                            nc.tensor.matmul(sA[:, osl], lhs, qtp[g][0][:, qsl],
                                             start=True, stop=True)
                            nc.tensor.matmul(sB[:, osl], lhs, qtp[g][1][:, qsl],
                                             start=True, stop=True)
                        eA = expp.tile([128, ns * 512], BF16, tag="eA",
                                       name=f"eA{g}_{qc}_{kc0}")
                        ei = nc.scalar.activation(out=eA, in_=sA, func=AF.Exp,
                                                  scale=0.125)
                        for ln in ln_insts:
                            add_dep_helper(ei.ins, ln.ins, False,
                                           "attn Exp after Ln block")
                        eB = expp.tile([128, ns * 512], BF16, tag="eB",
                                       name=f"eB{g}_{qc}_{kc0}")
                        ei = nc.scalar.activation(out=eB, in_=sB, func=AF.Exp,
                                                  scale=0.125)
                        for ln in ln_insts:
                            add_dep_helper(ei.ins, ln.ins, False,
                                           "attn Exp after Ln block")
                        for kk in range(ns):
                            kc = kc0 + kk
                            osl = slice(kk * 512, (kk + 1) * 512)
                            nc.tensor.matmul(cA, vp[g * 2][:, kc * 65:(kc + 1) * 65],
                                             eA[:, osl],
                                             start=(kc == 0), stop=(kc == KC - 1))
                            nc.tensor.matmul(cB, vp[g * 2 + 1][:, kc * 65:(kc + 1) * 65],
                                             eB[:, osl],
                                             start=(kc == 0), stop=(kc == KC - 1))
                    # epilogue: transpose ctxT back, normalize, store
                    csA = ep.tile([65, 512], BF16, tag="csA", name=f"csA{g}_{qc}")
                    nc.vector.tensor_copy(csA, cA)
                    csB = ep.tile([65, 512], BF16, tag="csB", name=f"csB{g}_{qc}")
                    nc.vector.tensor_copy(csB, cB)
                    for j in range(4):
                        jsl = slice(j * 128, (j + 1) * 128)
                        ptA = ps_c.tile([128, 65], BF16, tag="cA",
                                        name=f"ptA{g}_{qc}_{j}")
                        nc.tensor.transpose(ptA, csA[:, jsl], identb[0:65, 0:65])
                        ptB = ps_c.tile([128, 65], BF16, tag="cB",
                                        name=f"ptB{g}_{qc}_{j}")
                        nc.tensor.transpose(ptB, csB[:, jsl], identb[0:65, 0:65])
                        rA = ep.tile([128, 1], F32, tag="rA", name=f"rA{g}_{qc}_{j}")
                        nc.vector.reciprocal(rA, ptA[:, 64:65])
                        rB = ep.tile([128, 1], F32, tag="rB", name=f"rB{g}_{qc}_{j}")
                        nc.vector.reciprocal(rB, ptB[:, 64:65])
                        cf = ep.tile([128, 128], F32, tag="cf", name=f"cf{g}_{qc}_{j}")
                        nc.vector.tensor_scalar_mul(cf[:, 0:64], ptA[:, 0:64], rA)
                        nc.vector.tensor_scalar_mul(cf[:, 64:128], ptB[:, 0:64], rB)
                        row = g * S + qc * 512 + j * 128
                        eng = nc.sync if (qc + j) % 2 == 0 else nc.gpsimd
                        eng.dma_start(out=out[row:row + 128, :], in_=cf)

            vts_hist = []
            te_hist = []
            ln_insts = []
            for g in range(NG):
                proj_group(g, vts_hist)
            emit_vtrans(*vts_hist[-1])
            # Ln block: kt = ln(te + 1). The tile scheduler would otherwise
            # interleave these with the Exp acts (one ACT_TABLE_LOAD per
            # Exp<->Ln boundary, 1.3us each); nosync deps pin the phase
            # order Exp-block -> Ln-block -> attention-Exp-block.
            for g, sc, te, _ in te_hist:
                ln = nc.scalar.activation(out=kt[g][:, sc * 512:(sc + 1) * 512],
                                          in_=te, func=AF.Ln, bias=1.0)
                for _, _, _, tei in te_hist:
                    add_dep_helper(ln.ins, tei.ins, False,
                                   "Ln block after all te Exps")
                ln_insts.append(ln)
            for g in range(NG):
                attn_group(g)

    nc.finalize()
    return nc


def _get_nc():
    if "nc" not in _CACHE:
        _CACHE["nc"] = _build()
    return _CACHE["nc"]


def _shard_inputs(hidden_states, attention_mask, Wq, bq, Wk, bk):
    bf16 = ml_dtypes.bfloat16
    hs = np.asarray(hidden_states, dtype=np.float32)
    am = np.asarray(attention_mask)
    Wq = np.asarray(Wq, dtype=np.float32)
    Wk = np.asarray(Wk, dtype=np.float32)
    bq = np.asarray(bq, dtype=np.float32)
    bk = np.asarray(bk, dtype=np.float32)

    xts = [np.ascontiguousarray(hs[b].T).astype(bf16) for b in range(B)]
    maskbs = [np.ascontiguousarray(am[b].astype(bf16)[None, :])
              for b in range(B)]

    in_maps = []
    for c in range(NCORES):
        b = c // (NCORES // B)
        hg = c % (NCORES // B)
        cols = slice(hg * 2 * 128, (hg + 1) * 2 * 128)

        def _tile_w(W):
            # [128, g*1024 + dc*128 + j] = W[dc*128 + p, cols[g*128 + j]]
            a = W[:, cols].reshape(8, 128, NG, 128).transpose(1, 2, 0, 3)
            return np.ascontiguousarray(a.reshape(128, NG * 8 * 128)).astype(bf16)

        in_maps.append({
            "xt": xts[b],
            "wq": _tile_w(Wq),
            "wk": _tile_w(Wk),
            "bq": np.ascontiguousarray(bq[cols]),
            "bk": np.ascontiguousarray(bk[cols]),
            "maskb": maskbs[b],
        })
    return in_maps


def _gather(results):
    full = np.empty((B, S, D), dtype=np.float32)
    for c in range(NCORES):
        b = c // (NCORES // B)
        hg = c % (NCORES // B)
        cols = slice(hg * 2 * 128, (hg + 1) * 2 * 128)
        r = results[c]["out"].reshape(NG, S, 128)
        full[b, :, cols] = np.concatenate([r[0], r[1]], axis=1)
    return full


def run_sharded(in_maps, **kw):
    from concourse.bass_utils import run_bass_kernel_spmd
    nc = _get_nc()
    return run_bass_kernel_spmd(nc, in_maps, list(range(NCORES)), **kw)


def kernel(hidden_states, attention_mask, Wq, bq, Wk, bk):
    in_maps = _shard_inputs(hidden_states, attention_mask, Wq, bq, Wk, bk)
    res = run_sharded(in_maps)
    return _gather(res.results)
